# revision 21
# baseline (speedup 1.0000x reference)
"""Trainium2 Bass kernel for nn_Atten_RNN: fp16 weights/activations on the
matmul paths (fp32 PSUM accumulation + fp32 softmax/bias), PE-based scores
and contiguous-DVE attention on an SBUF-resident h history.

Sharding: batch-parallel (B=64 -> 8 per core) for RNN + attention; vocab-
parallel (32000 -> 4000 per core) for W_out, joined by one feat AllGather.

RNN is weight-stationary in the h^T layout: per step 64 fp16 LDW+matmul pairs
(K=128, M=128, N=8) accumulate pre^T in one PSUM bank, then one DVE add
(x-projection, fp32) and one tanh -> hT (fp16) directly. The h history is
kept in SBUF as o2v[p, mc, b, s] (h-on-partition, s innermost) so attention
needs no DRAM round trip:
  scores: per (b, mc) 1-col-stationary matmuls, K=128, N=512, accum over mc.
  att:    per mc DVE mul by broadcast attn + contiguous reduce over s.
"""

import numpy as np
from contextlib import ExitStack

import concourse.bass as bass
import concourse.tile as tile
from concourse import bacc, mybir
from concourse.bass_utils import run_bass_kernel_spmd
from concourse.masks import make_identity

FP = mybir.dt.float32
F16 = mybir.dt.float16
I16 = mybir.dt.int16

N_CORES = 8
B = 64
B_LOC = B // N_CORES          # 8
S_FULL = 512
E = 512
H = 1024
V = 32000
V_SH = V // N_CORES           # 4000
KC = H // 128                 # 8 hidden chunks
QC = E // 128                 # 4 embedding chunks
WO_PRE = 6                    # W_out k-slices prefetched during the RNN
WO_PRE_SLICES = [8, 9, 10, 11, 12, 13]  # last-half weights are used first
Tanh = mybir.ActivationFunctionType.Tanh
Exp = mybir.ActivationFunctionType.Exp


def build_nc(S=S_FULL, n_cores=N_CORES, collective=True):
    NT = S * B_LOC            # tokens per core, t = s*B_LOC + b
    assert S % 128 == 0 and NT % 128 == 0

    nc = bacc.Bacc("TRN2", target_bir_lowering=False, debug=False,
                   num_devices=n_cores)

    # ---- external I/O (per core) ----
    x_idx = nc.dram_tensor("x_idx", [128, NT // 16], I16, kind="ExternalInput")
    emb_t = nc.dram_tensor("emb_t", [V, E], F16, kind="ExternalInput")
    w_ihT = nc.dram_tensor("w_ihT", [QC, 128, H], F16, kind="ExternalInput")
    w_hhT = nc.dram_tensor("w_hhT", [KC, 128, H], F16, kind="ExternalInput")
    bias_pb = nc.dram_tensor("bias_pb", [128, KC], FP, kind="ExternalInput")
    w_outT = nc.dram_tensor("w_outT", [16, 128, V_SH], F16, kind="ExternalInput")
    b_out_sh = nc.dram_tensor("b_out_sh", [1, V_SH], F16, kind="ExternalInput")
    y_out = nc.dram_tensor("y_out", [B, V_SH], FP, kind="ExternalOutput")

    # ---- internal DRAM ----
    xw_dram = nc.dram_tensor("xw_dram", [KC, NT // 512, 128, 512], F16)
    attn_dram = nc.dram_tensor("attn_dram", [1, B_LOC * S], F16)
    scores_dram = nc.dram_tensor("scores_dram", [1, B_LOC * S], FP)
    agl_in = nc.dram_tensor("agl_in", [128, KC * B_LOC], F16)
    agl_out = nc.dram_tensor("agl_out", [N_CORES * 128, KC * B_LOC], F16,
                             addr_space="Shared")
    aga_in = nc.dram_tensor("aga_in", [128, KC * B_LOC], F16)
    aga_out = nc.dram_tensor("aga_out", [N_CORES * 128, KC * B_LOC], F16,
                             addr_space="Shared")

    with tile.TileContext(nc) as tc, ExitStack() as top:
        consts = top.enter_context(tc.tile_pool(name="consts", bufs=1))
        ident = consts.tile([128, 128], F16)
        make_identity(nc, ident[:])
        ones_row = consts.tile([1, 128], F16)
        nc.vector.memset(ones_row[:], 1.0)
        bias_pb_sb = consts.tile([128, KC], FP)
        nc.sync.dma_start(bias_pb_sb[:], bias_pb[:, :])
        bout_sb = consts.tile([1, V_SH], F16)
        nc.sync.dma_start(bout_sb[0:1, :], b_out_sh[0:1, :])
        lastT_sb = consts.tile([128, KC * B_LOC], F16)

        # attention SBUF pool (small tiles; outlives o2v, used through proj)
        att_sb = top.enter_context(tc.tile_pool(name="att_sb", bufs=1))

        # ================= phase 1+2: gather + transpose -> xeT =================
        with tc.tile_pool(name="xeT_p", bufs=1) as xeT_p, \
             tc.tile_pool(name="wih_p", bufs=1) as wih_p:
            wih_sb = wih_p.tile([128, QC, H], F16)
            nc.sync.dma_start(wih_sb[:], w_ihT.ap().rearrange("q p h -> p q h"))
            xeT_sb = xeT_p.tile([128, QC, NT], F16)
            with tc.tile_pool(name="xe_p", bufs=1) as xe_p, \
                 tc.tile_pool(name="idx_p", bufs=1) as idx_p, \
                 tc.tile_pool(name="trp_p", bufs=4, space="PSUM") as trp_p:
                xidx_sb = idx_p.tile([128, NT // 16], I16)
                nc.sync.dma_start(xidx_sb[:], x_idx[:, :])
                xe_sb = xe_p.tile([128, NT // 128, E], F16)
                GCH = 1024  # indices per dma_gather (keeps SWDGE ring within capacity)
                for g in range(NT // GCH):
                    nc.gpsimd.dma_gather(
                        out_ap=xe_sb[:, g * (GCH // 128):(g + 1) * (GCH // 128), :],
                        in_ap=emb_t.ap(),
                        idxs_ap=xidx_sb[:, g * (GCH // 16):(g + 1) * (GCH // 16)],
                        num_idxs=GCH, num_idxs_reg=GCH, elem_size=E)
                for c in range(NT // 128):
                    for q in range(QC):
                        pt = trp_p.tile([128, 128], F16)
                        nc.tensor.transpose(pt[:], xe_sb[:, c, q * 128:(q + 1) * 128], ident[:])
                        nc.vector.tensor_copy(xeT_sb[:, q, c * 128:(c + 1) * 128], pt[:])

            # ========= phase 3: xW^T[h, t] = W_ih @ xe^T + bias (hT layout) =========
            with tc.tile_pool(name="xw_ps", bufs=2, space="PSUM") as xw_ps, \
                 tc.tile_pool(name="xw_ev", bufs=3) as xw_ev:
                for hc in range(KC):
                    for tcn in range(NT // 512):
                        ps = xw_ps.tile([128, 512], FP)
                        for q in range(QC):
                            nc.tensor.matmul(
                                ps[:], wih_sb[:, q, hc * 128:(hc + 1) * 128],
                                xeT_sb[:, q, tcn * 512:(tcn + 1) * 512],
                                start=(q == 0), stop=(q == QC - 1))
                        ev = xw_ev.tile([128, 512], F16)
                        nc.vector.tensor_scalar_add(ev[:], ps[:], bias_pb_sb[:, hc:hc + 1])
                        nc.sync.dma_start(xw_dram[hc, tcn, :, :], ev[:])

        # W_out prefetch buffer (first WO_PRE k-slices, loaded during RNN)
        woA_p = top.enter_context(tc.tile_pool(name="woA_p", bufs=1))
        woA = woA_p.tile([128, WO_PRE, V_SH], F16)

        # h history, h-on-partition, s innermost: o2v[p, mc, b, s]
        # (scoped: freed before the projection phase needs its SBUF)
        o2v_stack = ExitStack()
        o2v_p = o2v_stack.enter_context(tc.tile_pool(name="o2v_p", bufs=1))
        o2v = o2v_p.tile([128, KC, B_LOC, S], F16)

        # W_out prefetch: issue early; DMA engines are idle during the RNN.
        for i, k in enumerate(WO_PRE_SLICES):
            nc.sync.dma_start(woA[:, i, :], w_outT[k, :, :])

        # ================= phase 4: RNN (weight-stationary, hT layout) =========
        # Wavefront (qf, kc-pair) emission: psum quarter 0 finishes early so
        # its tanh overlaps the tail of the step; consumers touch late h
        # quarters late, so the tanh chain pipelines across steps with no
        # exposed PE stall. tanh writes straight into o2v (no stage buffer).
        QW = 2 * B_LOC                # 16 cols per quarter
        WAVE = [(0, 0), (1, 0), (0, 1), (2, 0), (1, 1), (0, 2), (3, 0),
                (2, 1), (1, 2), (0, 3), (3, 1), (2, 2), (1, 3), (3, 2),
                (2, 3), (3, 3)]
        with tc.tile_pool(name="whh_p", bufs=1) as whh_p, \
             tc.tile_pool(name="h0_p", bufs=1) as h0_p, \
             tc.tile_pool(name="xwb_p", bufs=2) as xwb_p, \
             tc.tile_pool(name="rnn_ps", bufs=8, space="PSUM") as rnn_ps:
            whh_sb = whh_p.tile([128, KC, H], F16)
            nc.sync.dma_start(whh_sb[:], w_hhT.ap().rearrange("k p h -> p k h"))
            h0 = h0_p.tile([128, KC, B_LOC], F16)
            nc.vector.memset(h0[:], 0.0)
            prevq = [h0[:, q * 2:(q + 1) * 2, :] for q in range(4)]
            xwb = None
            for s in range(S):
                if s % 64 == 0:
                    blk = s // 64
                    xwb = xwb_p.tile([128, KC, 512], F16, tag="xwb")
                    nc.sync.dma_start(xwb[:], xw_dram[:, blk, :, :].rearrange(
                        "k p t -> p k t"))
                xw_sl = xwb[:].rearrange("p k (si b) -> p si k b", b=B_LOC)
                ph = [None] * 4
                for qf, kp in WAVE:
                    if kp == 0:
                        ph[qf] = rnn_ps.tile([128, QW], FP, tag="rnn_psum",
                                             name=f"ph_{s}_{qf}")
                        nc.tensor.matmul(
                            ph[qf][:], ident[:],
                            xw_sl[:, s % 64, qf * 2:(qf + 1) * 2, :],
                            start=True, stop=False, skip_group_check=True)
                    hp_src = prevq[kp]
                    for kc in (kp * 2, kp * 2 + 1):
                        for mc in (qf * 2, qf * 2 + 1):
                            nc.tensor.matmul(
                                ph[qf][:, (mc - qf * 2) * B_LOC:(mc - qf * 2 + 1) * B_LOC],
                                whh_sb[:, kc, mc * 128:(mc + 1) * 128],
                                hp_src[:, kc % 2, :],
                                start=False,
                                stop=(kp == 3 and kc == kp * 2 + 1 and mc == qf * 2 + 1),
                                skip_group_check=True)
                    if kp == 3:
                        nc.scalar.activation(
                            o2v[:, qf * 2:(qf + 1) * 2, :, s],
                            ph[qf][:].rearrange("p (m b) -> p m b", b=B_LOC), Tanh)
                prevq = [o2v[:, q * 2:(q + 1) * 2, :, s] for q in range(4)]
            nc.vector.tensor_copy(
                lastT_sb[:].rearrange("p (m b) -> p m b", b=B_LOC),
                o2v[:, :, :, S - 1])
        nc.sync.dma_start(agl_in[:, :], lastT_sb[:])
        if collective:
            nc.gpsimd.collective_compute(
                "AllGather", mybir.AluOpType.bypass,
                replica_groups=[list(range(n_cores))],
                ins=[agl_in.ap()], outs=[agl_out.ap()])
        else:
            for cc in range(n_cores):
                nc.sync.dma_start(agl_out[cc * 128:(cc + 1) * 128, :], agl_in[:, :])

        # ================= phase 5: attention =================
        with tc.tile_pool(name="att_ps", bufs=2, space="PSUM") as att_ps, \
             tc.tile_pool(name="sc_ps", bufs=4, space="PSUM") as sc_ps:
            # scores[b, s] = sum_h h_s[b,h]*last[b,h]: per (b, mc) 1-col matmuls
            scoresT = att_sb.tile([B_LOC, S], FP)
            with tc.tile_pool(name="scrow_p", bufs=2) as scrow_p:
                for b in range(B_LOC):
                    ps = sc_ps.tile([1, S], FP, tag="sc")
                    for mc in range(KC):
                        nc.tensor.matmul(
                            ps[:], lastT_sb[:, mc * B_LOC + b:mc * B_LOC + b + 1],
                            o2v[:, mc, b, :],
                            start=(mc == 0), stop=(mc == KC - 1))
                    row = scrow_p.tile([1, S], FP, tag="scrow")
                    nc.vector.tensor_copy(row[:], ps[:])
                    nc.sync.dma_start(scores_dram[0:1, b * S:(b + 1) * S], row[:])
            nc.sync.dma_start(
                scoresT[:, :],
                scores_dram.ap().rearrange("o (b s) -> (o b) s", b=B_LOC))

            # softmax over time (b on partitions); step S-1 excluded
            nc.vector.memset(scoresT[:, S - 1:S], -1e30)
            negmax = att_sb.tile([B_LOC, 1], FP)
            nc.vector.reduce_max(negmax[:], scoresT[:], axis=mybir.AxisListType.X, negate=True)
            expT = att_sb.tile([B_LOC, S], FP)
            nc.scalar.activation(expT[:], scoresT[:], Exp, bias=negmax[:])
            ssum = att_sb.tile([B_LOC, 1], FP)
            nc.vector.reduce_sum(ssum[:], expT[:], axis=mybir.AxisListType.X)
            rinv = att_sb.tile([B_LOC, 1], FP)
            nc.vector.reciprocal(rinv[:], ssum[:])
            attnT = att_sb.tile([B_LOC, S], F16)
            nc.vector.tensor_scalar_mul(attnT[:], expT[:], rinv[:])

            # broadcast attn over partitions: attnB[p, (b, s)] fp16
            nc.sync.dma_start(
                attn_dram.ap().rearrange("o (b s) -> (o b) s", b=B_LOC), attnT[:])
            attnB = att_sb.tile([128, B_LOC * S], F16)
            with tc.tile_pool(name="arow_p", bufs=2) as arow_p:
                for i in range(B_LOC * S // 512):
                    ar = arow_p.tile([1, 512], F16, tag="arow")
                    nc.sync.dma_start(ar[:], attn_dram[0:1, i * 512:(i + 1) * 512])
                    ab = att_ps.tile([128, 512], FP, tag="attps")
                    nc.tensor.matmul(ab[:], ones_row[:, 0:128], ar[:],
                                     start=True, stop=True)
                    nc.vector.tensor_copy(attnB[:, i * 512:(i + 1) * 512], ab[:])

            # att[b, h=mc*128+p]: per mc DVE mul + contiguous reduce over s
            att_acc = att_sb.tile([128, KC, B_LOC], FP)
            with tc.tile_pool(name="prod_p", bufs=2) as prod_p:
                for mc in range(KC):
                    pr = prod_p.tile([128, B_LOC * S], F16, tag="prod")
                    nc.vector.tensor_mul(pr[:], o2v[:, mc, :, :], attnB[:])
                    nc.vector.reduce_sum(
                        att_acc[:, mc, :],
                        pr[:].rearrange("p (b s) -> p b s", b=B_LOC),
                        axis=mybir.AxisListType.X)
            att16 = att_sb.tile([128, KC * B_LOC], F16)
            nc.vector.tensor_copy(
                att16[:].rearrange("p (m b) -> p m b", b=B_LOC), att_acc[:])
            nc.sync.dma_start(aga_in[:, :], att16[:])
        o2v_stack.close()
        if collective:
            nc.gpsimd.collective_compute(
                "AllGather", mybir.AluOpType.bypass,
                replica_groups=[list(range(n_cores))],
                ins=[aga_in.ap()], outs=[aga_out.ap()])
        else:
            for cc in range(n_cores):
                nc.sync.dma_start(aga_out[cc * 128:(cc + 1) * 128, :], aga_in[:, :])

        # ================= phase 6: projection =================
        # featT halves come straight from the gathered transposed layouts.
        NV = V_SH // 8  # 500-wide psum chunks
        with tc.tile_pool(name="fT_p", bufs=1) as fT_p, \
             tc.tile_pool(name="wo_p", bufs=4) as wo_p, \
             tc.tile_pool(name="y_ps", bufs=1, space="PSUM") as y_ps, \
             tc.tile_pool(name="y_sb_p", bufs=1) as y_sb_p:
            featT_l = fT_p.tile([128, KC, N_CORES, B_LOC], F16)
            nc.sync.dma_start(
                featT_l[:],
                agl_out.ap().rearrange("(c p) (m b) -> p m c b",
                                       p=128, m=KC, b=B_LOC))
            featT_a = fT_p.tile([128, KC, N_CORES, B_LOC], F16)
            nc.sync.dma_start(
                featT_a[:],
                aga_out.ap().rearrange("(c p) (m b) -> p m c b",
                                       p=128, m=KC, b=B_LOC))
            psums = [y_ps.tile([B, NV], FP, tag=f"y{n}", name=f"ypsum{n}")
                     for n in range(8)]
            # last half first (kc 8..15): ready after CC1, overlaps attention
            for kc in list(range(8, 16)) + list(range(8)):
                if kc in WO_PRE_SLICES:
                    wot = woA[:, WO_PRE_SLICES.index(kc), :]
                else:
                    wt = wo_p.tile([128, V_SH], F16, tag="wot")
                    nc.sync.dma_start(wt[:], w_outT[kc, :, :])
                    wot = wt[:]
                fT = (featT_l if kc >= 8 else featT_a)[
                    :, kc - 8 if kc >= 8 else kc, :, :].rearrange(
                    "p c b -> p (c b)")
                for n in range(8):
                    nc.tensor.matmul(psums[n][:], fT,
                                     wot[:, n * NV:(n + 1) * NV],
                                     start=(kc == 8), stop=False)
            for n in range(8):
                nc.tensor.matmul(psums[n][:], ones_row[:, 0:B],
                                 bout_sb[0:1, n * NV:(n + 1) * NV],
                                 start=False, stop=True)
            y_sb = y_sb_p.tile([B, V_SH], FP)
            for n in range(8):
                nc.vector.tensor_copy(y_sb[:, n * NV:(n + 1) * NV], psums[n][:])
            nc.sync.dma_start(y_out[:, :], y_sb[:])

    nc.compile()
    return nc


def host_prep(X, emb, W_ih, W_hh, b_ih, b_hh, W_out, b_out, S=S_FULL, n_cores=N_CORES):
    """Build the per-core input maps (sharding + fp16 layout prep on host)."""
    NT = S * B_LOC
    emb_f = np.ascontiguousarray(np.asarray(emb, np.float32).astype(np.float16))
    w_ihT = np.ascontiguousarray(
        np.asarray(W_ih, np.float32).T.astype(np.float16).reshape(QC, 128, H))
    w_hhT = np.ascontiguousarray(
        np.asarray(W_hh, np.float32).T.astype(np.float16).reshape(KC, 128, H))
    bias_pb = np.ascontiguousarray(
        (np.asarray(b_ih, np.float32) + np.asarray(b_hh, np.float32)).reshape(KC, 128).T)
    in_maps = []
    for c in range(n_cores):
        Xl = np.asarray(X[c * B_LOC:(c + 1) * B_LOC, :S])
        tok = Xl.T.reshape(-1)                        # t = s*B_LOC + b
        idx = np.zeros((128, NT // 16), np.int16)
        for g in range(8):
            idx[g * 16:(g + 1) * 16, :] = tok.reshape(NT // 16, 16).T
        Wo = np.asarray(W_out[c * V_SH:(c + 1) * V_SH, :], np.float32)
        w_outT = np.ascontiguousarray(Wo.T.astype(np.float16).reshape(16, 128, V_SH))
        in_maps.append({
            "x_idx": idx,
            "emb_t": emb_f,
            "w_ihT": w_ihT,
            "w_hhT": w_hhT,
            "bias_pb": bias_pb,
            "w_outT": w_outT,
            "b_out_sh": np.asarray(b_out[c * V_SH:(c + 1) * V_SH],
                                   np.float32).astype(np.float16).reshape(1, V_SH),
        })
    return in_maps


_NC_CACHE = {}


def kernel(X, emb, W_ih, W_hh, b_ih, b_hh, W_out, b_out):
    X = np.asarray(X)
    in_maps = host_prep(X, emb, W_ih, W_hh, b_ih, b_hh, W_out, b_out)
    if "nc" not in _NC_CACHE:
        _NC_CACHE["nc"] = build_nc()
    nc = _NC_CACHE["nc"]
    res = run_bass_kernel_spmd(nc, in_maps, list(range(N_CORES)))
    Y = np.concatenate([res.results[i]["y_out"] for i in range(N_CORES)], axis=1)
    return Y.astype(np.float32)


if __name__ == "__main__":
    import importlib.util
    spec = importlib.util.spec_from_file_location("reference", "/root/problem/reference.py")
    ref = importlib.util.module_from_spec(spec)
    spec.loader.exec_module(ref)
    inputs = {k: np.asarray(v) for k, v in ref.setup_inputs().items()}
    Y = kernel(**inputs)
    print(Y.shape, Y.dtype)


# revision 22
# speedup vs baseline: 1.0362x; 1.0362x over previous
"""Trainium2 Bass kernel for nn_Atten_RNN: fp16 weights/activations on the
matmul paths (fp32 PSUM accumulation + fp32 softmax/bias), PE-based scores
and contiguous-DVE attention on an SBUF-resident h history.

Sharding: batch-parallel (B=64 -> 8 per core) for RNN + attention; vocab-
parallel (32000 -> 4000 per core) for W_out, joined by one feat AllGather.

RNN is weight-stationary in the h^T layout: per step 64 fp16 LDW+matmul pairs
(K=128, M=128, N=8) accumulate pre^T in one PSUM bank, then one DVE add
(x-projection, fp32) and one tanh -> hT (fp16) directly. The h history is
kept in SBUF as o2v[p, mc, b, s] (h-on-partition, s innermost) so attention
needs no DRAM round trip:
  scores: per (b, mc) 1-col-stationary matmuls, K=128, N=512, accum over mc.
  att:    per mc DVE mul by broadcast attn + contiguous reduce over s.
"""

import numpy as np
from contextlib import ExitStack

import concourse.bass as bass
import concourse.tile as tile
from concourse.tile import add_dep_helper
from concourse import bacc, mybir
from concourse.bass_utils import run_bass_kernel_spmd
from concourse.masks import make_identity

FP = mybir.dt.float32
F16 = mybir.dt.float16
I16 = mybir.dt.int16

N_CORES = 8
B = 64
B_LOC = B // N_CORES          # 8
S_FULL = 512
E = 512
H = 1024
V = 32000
V_SH = V // N_CORES           # 4000
KC = H // 128                 # 8 hidden chunks
QC = E // 128                 # 4 embedding chunks
WO_PRE = 6                    # W_out k-slices prefetched during the RNN
WO_PRE_SLICES = [8, 9, 10, 11, 12, 13]  # last-half weights are used first
Tanh = mybir.ActivationFunctionType.Tanh
Exp = mybir.ActivationFunctionType.Exp


def build_nc(S=S_FULL, n_cores=N_CORES, collective=True):
    NT = S * B_LOC            # tokens per core, t = s*B_LOC + b
    assert S % 128 == 0 and NT % 128 == 0

    nc = bacc.Bacc("TRN2", target_bir_lowering=False, debug=False,
                   num_devices=n_cores)

    # ---- external I/O (per core) ----
    x_idx = nc.dram_tensor("x_idx", [128, NT // 16], I16, kind="ExternalInput")
    emb_t = nc.dram_tensor("emb_t", [V, E], F16, kind="ExternalInput")
    w_ihT = nc.dram_tensor("w_ihT", [QC, 128, H], F16, kind="ExternalInput")
    w_hhT = nc.dram_tensor("w_hhT", [KC, 128, H], F16, kind="ExternalInput")
    bias_pb = nc.dram_tensor("bias_pb", [128, KC], FP, kind="ExternalInput")
    w_outT = nc.dram_tensor("w_outT", [16, 128, V_SH], F16, kind="ExternalInput")
    b_out_sh = nc.dram_tensor("b_out_sh", [1, V_SH], F16, kind="ExternalInput")
    y_out = nc.dram_tensor("y_out", [B, V_SH], FP, kind="ExternalOutput")

    # ---- internal DRAM ----
    xw_dram = nc.dram_tensor("xw_dram", [KC, NT // 512, 128, 512], F16)
    attn_dram = nc.dram_tensor("attn_dram", [1, B_LOC * S], F16)
    scores_dram = nc.dram_tensor("scores_dram", [1, B_LOC * S], FP)
    agl_in = nc.dram_tensor("agl_in", [128, KC * B_LOC], F16)
    agl_out = nc.dram_tensor("agl_out", [N_CORES * 128, KC * B_LOC], F16,
                             addr_space="Shared")
    aga_in = nc.dram_tensor("aga_in", [128, KC * B_LOC], F16)
    aga_out = nc.dram_tensor("aga_out", [N_CORES * 128, KC * B_LOC], F16,
                             addr_space="Shared")

    with tile.TileContext(nc) as tc, ExitStack() as top:
        consts = top.enter_context(tc.tile_pool(name="consts", bufs=1))
        ident = consts.tile([128, 128], F16)
        make_identity(nc, ident[:])
        ones_row = consts.tile([1, 128], F16)
        nc.vector.memset(ones_row[:], 1.0)
        bias_pb_sb = consts.tile([128, KC], FP)
        nc.sync.dma_start(bias_pb_sb[:], bias_pb[:, :])
        bout_sb = consts.tile([1, V_SH], F16)
        nc.sync.dma_start(bout_sb[0:1, :], b_out_sh[0:1, :])
        lastT_sb = consts.tile([128, KC * B_LOC], F16)

        # attention SBUF pool (small tiles; outlives o2v, used through proj)
        att_sb = top.enter_context(tc.tile_pool(name="att_sb", bufs=1))

        # ================= phase 1+2: gather + transpose -> xeT =================
        with tc.tile_pool(name="xeT_p", bufs=1) as xeT_p, \
             tc.tile_pool(name="wih_p", bufs=1) as wih_p:
            wih_sb = wih_p.tile([128, QC, H], F16)
            nc.sync.dma_start(wih_sb[:], w_ihT.ap().rearrange("q p h -> p q h"))
            xeT_sb = xeT_p.tile([128, QC, NT], F16)
            with tc.tile_pool(name="xe_p", bufs=1) as xe_p, \
                 tc.tile_pool(name="idx_p", bufs=1) as idx_p, \
                 tc.tile_pool(name="trp_p", bufs=4, space="PSUM") as trp_p:
                xidx_sb = idx_p.tile([128, NT // 16], I16)
                nc.sync.dma_start(xidx_sb[:], x_idx[:, :])
                xe_sb = xe_p.tile([128, NT // 128, E], F16)
                GCH = 1024  # indices per dma_gather (keeps SWDGE ring within capacity)
                for g in range(NT // GCH):
                    nc.gpsimd.dma_gather(
                        out_ap=xe_sb[:, g * (GCH // 128):(g + 1) * (GCH // 128), :],
                        in_ap=emb_t.ap(),
                        idxs_ap=xidx_sb[:, g * (GCH // 16):(g + 1) * (GCH // 16)],
                        num_idxs=GCH, num_idxs_reg=GCH, elem_size=E)
                for c in range(NT // 128):
                    for q in range(QC):
                        pt = trp_p.tile([128, 128], F16)
                        nc.tensor.transpose(pt[:], xe_sb[:, c, q * 128:(q + 1) * 128], ident[:])
                        nc.vector.tensor_copy(xeT_sb[:, q, c * 128:(c + 1) * 128], pt[:])

            # ========= phase 3: xW^T[h, t] = W_ih @ xe^T + bias (hT layout) =========
            with tc.tile_pool(name="xw_ps", bufs=2, space="PSUM") as xw_ps, \
                 tc.tile_pool(name="xw_ev", bufs=3) as xw_ev:
                for hc in range(KC):
                    for tcn in range(NT // 512):
                        ps = xw_ps.tile([128, 512], FP)
                        for q in range(QC):
                            nc.tensor.matmul(
                                ps[:], wih_sb[:, q, hc * 128:(hc + 1) * 128],
                                xeT_sb[:, q, tcn * 512:(tcn + 1) * 512],
                                start=(q == 0), stop=(q == QC - 1))
                        ev = xw_ev.tile([128, 512], F16)
                        nc.vector.tensor_scalar_add(ev[:], ps[:], bias_pb_sb[:, hc:hc + 1])
                        nc.sync.dma_start(xw_dram[hc, tcn, :, :], ev[:])

        # W_out prefetch buffer (first WO_PRE k-slices, loaded during RNN)
        woA_p = top.enter_context(tc.tile_pool(name="woA_p", bufs=1))
        woA = woA_p.tile([128, WO_PRE, V_SH], F16)

        # h history, h-on-partition, s innermost: o2v[p, mc, b, s]
        # (scoped: freed before the projection phase needs its SBUF)
        o2v_stack = ExitStack()
        o2v_p = o2v_stack.enter_context(tc.tile_pool(name="o2v_p", bufs=1))
        o2v = o2v_p.tile([128, KC, B_LOC, S], F16)

        # W_out prefetch: issue early; DMA engines are idle during the RNN.
        for i, k in enumerate(WO_PRE_SLICES):
            nc.sync.dma_start(woA[:, i, :], w_outT[k, :, :])

        # ================= phase 4: RNN (weight-stationary, hT layout) =========
        # Wavefront (qf, kc-pair) emission: psum quarter 0 finishes early so
        # its tanh overlaps the tail of the step; consumers touch late h
        # quarters late, so the tanh chain pipelines across steps with no
        # exposed PE stall. tanh writes straight into o2v (no stage buffer).
        QW = 2 * B_LOC                # 16 cols per quarter
        WAVE = [(0, 0), (1, 0), (0, 1), (2, 0), (1, 1), (0, 2), (3, 0),
                (2, 1), (1, 2), (0, 3), (3, 1), (2, 2), (1, 3), (3, 2),
                (2, 3), (3, 3)]
        with tc.tile_pool(name="whh_p", bufs=1) as whh_p, \
             tc.tile_pool(name="h0_p", bufs=1) as h0_p, \
             tc.tile_pool(name="xwb_p", bufs=2) as xwb_p, \
             tc.tile_pool(name="rnn_ps", bufs=8, space="PSUM") as rnn_ps:
            whh_sb = whh_p.tile([128, KC, H], F16)
            nc.sync.dma_start(whh_sb[:], w_hhT.ap().rearrange("k p h -> p k h"))
            h0 = h0_p.tile([128, KC, B_LOC], F16)
            nc.vector.memset(h0[:], 0.0)
            prevq = [h0[:, q * 2:(q + 1) * 2, :] for q in range(4)]
            xwb = None
            prev_mm = None
            for s in range(S):
                if s % 64 == 0:
                    blk = s // 64
                    xwb = xwb_p.tile([128, KC, 512], F16, tag="xwb")
                    nc.sync.dma_start(xwb[:], xw_dram[:, blk, :, :].rearrange(
                        "k p t -> p k t"))
                xw_sl = xwb[:].rearrange("p k (si b) -> p si k b", b=B_LOC)
                ph = [None] * 4
                for qf, kp in WAVE:
                    if kp == 0:
                        ph[qf] = rnn_ps.tile([128, QW], FP, tag="rnn_psum",
                                             name=f"ph_{s}_{qf}")
                        m = nc.tensor.matmul(
                            ph[qf][:], ident[:],
                            xw_sl[:, s % 64, qf * 2:(qf + 1) * 2, :],
                            start=True, stop=False, skip_group_check=True)
                        if prev_mm is not None:
                            add_dep_helper(m.ins, prev_mm.ins, sync=False,
                                           reason="rnn wavefront order")
                        prev_mm = m
                    hp_src = prevq[kp]
                    for kc in (kp * 2, kp * 2 + 1):
                        for mc in (qf * 2, qf * 2 + 1):
                            m = nc.tensor.matmul(
                                ph[qf][:, (mc - qf * 2) * B_LOC:(mc - qf * 2 + 1) * B_LOC],
                                whh_sb[:, kc, mc * 128:(mc + 1) * 128],
                                hp_src[:, kc % 2, :],
                                start=False,
                                stop=(kp == 3 and kc == kp * 2 + 1 and mc == qf * 2 + 1),
                                skip_group_check=True)
                            add_dep_helper(m.ins, prev_mm.ins, sync=False,
                                           reason="rnn wavefront order")
                            prev_mm = m
                    if kp == 3:
                        nc.scalar.activation(
                            o2v[:, qf * 2:(qf + 1) * 2, :, s],
                            ph[qf][:].rearrange("p (m b) -> p m b", b=B_LOC), Tanh)
                prevq = [o2v[:, q * 2:(q + 1) * 2, :, s] for q in range(4)]
            nc.vector.tensor_copy(
                lastT_sb[:].rearrange("p (m b) -> p m b", b=B_LOC),
                o2v[:, :, :, S - 1])
        nc.sync.dma_start(agl_in[:, :], lastT_sb[:])
        if collective:
            nc.gpsimd.collective_compute(
                "AllGather", mybir.AluOpType.bypass,
                replica_groups=[list(range(n_cores))],
                ins=[agl_in.ap()], outs=[agl_out.ap()])
        else:
            for cc in range(n_cores):
                nc.sync.dma_start(agl_out[cc * 128:(cc + 1) * 128, :], agl_in[:, :])

        # ================= phase 5: attention =================
        with tc.tile_pool(name="att_ps", bufs=2, space="PSUM") as att_ps, \
             tc.tile_pool(name="sc_ps", bufs=4, space="PSUM") as sc_ps:
            # scores[b, s] = sum_h h_s[b,h]*last[b,h]: per (b, mc) 1-col matmuls
            scoresT = att_sb.tile([B_LOC, S], FP)
            with tc.tile_pool(name="scrow_p", bufs=2) as scrow_p:
                for b in range(B_LOC):
                    ps = sc_ps.tile([1, S], FP, tag="sc")
                    for mc in range(KC):
                        nc.tensor.matmul(
                            ps[:], lastT_sb[:, mc * B_LOC + b:mc * B_LOC + b + 1],
                            o2v[:, mc, b, :],
                            start=(mc == 0), stop=(mc == KC - 1))
                    row = scrow_p.tile([1, S], FP, tag="scrow")
                    nc.vector.tensor_copy(row[:], ps[:])
                    nc.sync.dma_start(scores_dram[0:1, b * S:(b + 1) * S], row[:])
            nc.sync.dma_start(
                scoresT[:, :],
                scores_dram.ap().rearrange("o (b s) -> (o b) s", b=B_LOC))

            # softmax over time (b on partitions); step S-1 excluded
            nc.vector.memset(scoresT[:, S - 1:S], -1e30)
            negmax = att_sb.tile([B_LOC, 1], FP)
            nc.vector.reduce_max(negmax[:], scoresT[:], axis=mybir.AxisListType.X, negate=True)
            expT = att_sb.tile([B_LOC, S], FP)
            nc.scalar.activation(expT[:], scoresT[:], Exp, bias=negmax[:])
            ssum = att_sb.tile([B_LOC, 1], FP)
            nc.vector.reduce_sum(ssum[:], expT[:], axis=mybir.AxisListType.X)
            rinv = att_sb.tile([B_LOC, 1], FP)
            nc.vector.reciprocal(rinv[:], ssum[:])
            attnT = att_sb.tile([B_LOC, S], F16)
            nc.vector.tensor_scalar_mul(attnT[:], expT[:], rinv[:])

            # broadcast attn over partitions: attnB[p, (b, s)] fp16
            nc.sync.dma_start(
                attn_dram.ap().rearrange("o (b s) -> (o b) s", b=B_LOC), attnT[:])
            attnB = att_sb.tile([128, B_LOC * S], F16)
            with tc.tile_pool(name="arow_p", bufs=2) as arow_p:
                for i in range(B_LOC * S // 512):
                    ar = arow_p.tile([1, 512], F16, tag="arow")
                    nc.sync.dma_start(ar[:], attn_dram[0:1, i * 512:(i + 1) * 512])
                    ab = att_ps.tile([128, 512], FP, tag="attps")
                    nc.tensor.matmul(ab[:], ones_row[:, 0:128], ar[:],
                                     start=True, stop=True)
                    nc.vector.tensor_copy(attnB[:, i * 512:(i + 1) * 512], ab[:])

            # att[b, h=mc*128+p]: per mc DVE mul + contiguous reduce over s
            att_acc = att_sb.tile([128, KC, B_LOC], FP)
            with tc.tile_pool(name="prod_p", bufs=2) as prod_p:
                for mc in range(KC):
                    pr = prod_p.tile([128, B_LOC * S], F16, tag="prod")
                    nc.vector.tensor_mul(pr[:], o2v[:, mc, :, :], attnB[:])
                    nc.vector.reduce_sum(
                        att_acc[:, mc, :],
                        pr[:].rearrange("p (b s) -> p b s", b=B_LOC),
                        axis=mybir.AxisListType.X)
            att16 = att_sb.tile([128, KC * B_LOC], F16)
            nc.vector.tensor_copy(
                att16[:].rearrange("p (m b) -> p m b", b=B_LOC), att_acc[:])
            nc.sync.dma_start(aga_in[:, :], att16[:])
        o2v_stack.close()
        if collective:
            nc.gpsimd.collective_compute(
                "AllGather", mybir.AluOpType.bypass,
                replica_groups=[list(range(n_cores))],
                ins=[aga_in.ap()], outs=[aga_out.ap()])
        else:
            for cc in range(n_cores):
                nc.sync.dma_start(aga_out[cc * 128:(cc + 1) * 128, :], aga_in[:, :])

        # ================= phase 6: projection =================
        # featT halves come straight from the gathered transposed layouts.
        NV = V_SH // 8  # 500-wide psum chunks
        with tc.tile_pool(name="fT_p", bufs=1) as fT_p, \
             tc.tile_pool(name="wo_p", bufs=4) as wo_p, \
             tc.tile_pool(name="y_ps", bufs=1, space="PSUM") as y_ps, \
             tc.tile_pool(name="y_sb_p", bufs=1) as y_sb_p:
            featT_l = fT_p.tile([128, KC, N_CORES, B_LOC], F16)
            nc.sync.dma_start(
                featT_l[:],
                agl_out.ap().rearrange("(c p) (m b) -> p m c b",
                                       p=128, m=KC, b=B_LOC))
            featT_a = fT_p.tile([128, KC, N_CORES, B_LOC], F16)
            nc.sync.dma_start(
                featT_a[:],
                aga_out.ap().rearrange("(c p) (m b) -> p m c b",
                                       p=128, m=KC, b=B_LOC))
            psums = [y_ps.tile([B, NV], FP, tag=f"y{n}", name=f"ypsum{n}")
                     for n in range(8)]
            # last half first (kc 8..15): ready after CC1, overlaps attention
            for kc in list(range(8, 16)) + list(range(8)):
                if kc in WO_PRE_SLICES:
                    wot = woA[:, WO_PRE_SLICES.index(kc), :]
                else:
                    wt = wo_p.tile([128, V_SH], F16, tag="wot")
                    nc.sync.dma_start(wt[:], w_outT[kc, :, :])
                    wot = wt[:]
                fT = (featT_l if kc >= 8 else featT_a)[
                    :, kc - 8 if kc >= 8 else kc, :, :].rearrange(
                    "p c b -> p (c b)")
                for n in range(8):
                    nc.tensor.matmul(psums[n][:], fT,
                                     wot[:, n * NV:(n + 1) * NV],
                                     start=(kc == 8), stop=False)
            for n in range(8):
                nc.tensor.matmul(psums[n][:], ones_row[:, 0:B],
                                 bout_sb[0:1, n * NV:(n + 1) * NV],
                                 start=False, stop=True)
            y_sb = y_sb_p.tile([B, V_SH], FP)
            for n in range(8):
                nc.vector.tensor_copy(y_sb[:, n * NV:(n + 1) * NV], psums[n][:])
            nc.sync.dma_start(y_out[:, :], y_sb[:])

    nc.compile()
    return nc


def host_prep(X, emb, W_ih, W_hh, b_ih, b_hh, W_out, b_out, S=S_FULL, n_cores=N_CORES):
    """Build the per-core input maps (sharding + fp16 layout prep on host)."""
    NT = S * B_LOC
    emb_f = np.ascontiguousarray(np.asarray(emb, np.float32).astype(np.float16))
    w_ihT = np.ascontiguousarray(
        np.asarray(W_ih, np.float32).T.astype(np.float16).reshape(QC, 128, H))
    w_hhT = np.ascontiguousarray(
        np.asarray(W_hh, np.float32).T.astype(np.float16).reshape(KC, 128, H))
    bias_pb = np.ascontiguousarray(
        (np.asarray(b_ih, np.float32) + np.asarray(b_hh, np.float32)).reshape(KC, 128).T)
    in_maps = []
    for c in range(n_cores):
        Xl = np.asarray(X[c * B_LOC:(c + 1) * B_LOC, :S])
        tok = Xl.T.reshape(-1)                        # t = s*B_LOC + b
        idx = np.zeros((128, NT // 16), np.int16)
        for g in range(8):
            idx[g * 16:(g + 1) * 16, :] = tok.reshape(NT // 16, 16).T
        Wo = np.asarray(W_out[c * V_SH:(c + 1) * V_SH, :], np.float32)
        w_outT = np.ascontiguousarray(Wo.T.astype(np.float16).reshape(16, 128, V_SH))
        in_maps.append({
            "x_idx": idx,
            "emb_t": emb_f,
            "w_ihT": w_ihT,
            "w_hhT": w_hhT,
            "bias_pb": bias_pb,
            "w_outT": w_outT,
            "b_out_sh": np.asarray(b_out[c * V_SH:(c + 1) * V_SH],
                                   np.float32).astype(np.float16).reshape(1, V_SH),
        })
    return in_maps


_NC_CACHE = {}


def kernel(X, emb, W_ih, W_hh, b_ih, b_hh, W_out, b_out):
    X = np.asarray(X)
    in_maps = host_prep(X, emb, W_ih, W_hh, b_ih, b_hh, W_out, b_out)
    if "nc" not in _NC_CACHE:
        _NC_CACHE["nc"] = build_nc()
    nc = _NC_CACHE["nc"]
    res = run_bass_kernel_spmd(nc, in_maps, list(range(N_CORES)))
    Y = np.concatenate([res.results[i]["y_out"] for i in range(N_CORES)], axis=1)
    return Y.astype(np.float32)


if __name__ == "__main__":
    import importlib.util
    spec = importlib.util.spec_from_file_location("reference", "/root/problem/reference.py")
    ref = importlib.util.module_from_spec(spec)
    spec.loader.exec_module(ref)
    inputs = {k: np.asarray(v) for k, v in ref.setup_inputs().items()}
    Y = kernel(**inputs)
    print(Y.shape, Y.dtype)


# revision 23
# speedup vs baseline: 1.0481x; 1.0114x over previous
"""Trainium2 Bass kernel for nn_Atten_RNN: fp16 weights/activations on the
matmul paths (fp32 PSUM accumulation + fp32 softmax/bias), PE-based scores
and contiguous-DVE attention on an SBUF-resident h history.

Sharding: batch-parallel (B=64 -> 8 per core) for RNN + attention; vocab-
parallel (32000 -> 4000 per core) for W_out, joined by one feat AllGather.

RNN is weight-stationary in the h^T layout: per step 64 fp16 LDW+matmul pairs
(K=128, M=128, N=8) accumulate pre^T in one PSUM bank, then one DVE add
(x-projection, fp32) and one tanh -> hT (fp16) directly. The h history is
kept in SBUF as o2v[p, mc, b, s] (h-on-partition, s innermost) so attention
needs no DRAM round trip:
  scores: per (b, mc) 1-col-stationary matmuls, K=128, N=512, accum over mc.
  att:    per mc DVE mul by broadcast attn + contiguous reduce over s.
"""

import numpy as np
from contextlib import ExitStack

import concourse.bass as bass
import concourse.tile as tile
from concourse.tile import add_dep_helper
from concourse import bacc, mybir
from concourse.bass_utils import run_bass_kernel_spmd
from concourse.masks import make_identity

FP = mybir.dt.float32
F16 = mybir.dt.float16
I16 = mybir.dt.int16

N_CORES = 8
B = 64
B_LOC = B // N_CORES          # 8
S_FULL = 512
E = 512
H = 1024
V = 32000
V_SH = V // N_CORES           # 4000
KC = H // 128                 # 8 hidden chunks
QC = E // 128                 # 4 embedding chunks
WO_PRE = 6                    # W_out k-slices prefetched during the RNN
WO_PRE_SLICES = [8, 9, 10, 11, 12, 13]  # last-half weights are used first
Tanh = mybir.ActivationFunctionType.Tanh
Exp = mybir.ActivationFunctionType.Exp


def build_nc(S=S_FULL, n_cores=N_CORES, collective=True):
    NT = S * B_LOC            # tokens per core, t = s*B_LOC + b
    assert S % 128 == 0 and NT % 128 == 0

    nc = bacc.Bacc("TRN2", target_bir_lowering=False, debug=False,
                   num_devices=n_cores)

    # ---- external I/O (per core) ----
    x_idx = nc.dram_tensor("x_idx", [128, NT // 16], I16, kind="ExternalInput")
    emb_t = nc.dram_tensor("emb_t", [V, E], F16, kind="ExternalInput")
    w_ihT = nc.dram_tensor("w_ihT", [QC, 128, H], F16, kind="ExternalInput")
    w_hhT = nc.dram_tensor("w_hhT", [KC, 128, H], F16, kind="ExternalInput")
    bias_pb = nc.dram_tensor("bias_pb", [128, KC], FP, kind="ExternalInput")
    w_outT = nc.dram_tensor("w_outT", [16, 128, V_SH], F16, kind="ExternalInput")
    b_out_sh = nc.dram_tensor("b_out_sh", [1, V_SH], F16, kind="ExternalInput")
    y_out = nc.dram_tensor("y_out", [B, V_SH], FP, kind="ExternalOutput")

    # ---- internal DRAM ----
    xw_dram = nc.dram_tensor("xw_dram", [KC, NT // 512, 128, 512], F16)
    attn_dram = nc.dram_tensor("attn_dram", [1, B_LOC * S], F16)
    scores_dram = nc.dram_tensor("scores_dram", [1, B_LOC * S], FP)
    agl_in = nc.dram_tensor("agl_in", [128, KC * B_LOC], F16)
    agl_out = nc.dram_tensor("agl_out", [N_CORES * 128, KC * B_LOC], F16,
                             addr_space="Shared")
    aga_in = nc.dram_tensor("aga_in", [128, KC * B_LOC], F16)
    aga_out = nc.dram_tensor("aga_out", [N_CORES * 128, KC * B_LOC], F16,
                             addr_space="Shared")

    with tile.TileContext(nc) as tc, ExitStack() as top:
        consts = top.enter_context(tc.tile_pool(name="consts", bufs=1))
        ident = consts.tile([128, 128], F16)
        make_identity(nc, ident[:])
        ones_row = consts.tile([1, 128], F16)
        nc.vector.memset(ones_row[:], 1.0)
        bias_pb_sb = consts.tile([128, KC], FP)
        nc.sync.dma_start(bias_pb_sb[:], bias_pb[:, :])
        bout_sb = consts.tile([1, V_SH], F16)
        nc.sync.dma_start(bout_sb[0:1, :], b_out_sh[0:1, :])
        lastT_sb = consts.tile([128, KC * B_LOC], F16)

        # attention SBUF pool (small tiles; outlives o2v, used through proj)
        att_sb = top.enter_context(tc.tile_pool(name="att_sb", bufs=1))

        # ================= phase 1+2: gather + transpose -> xeT =================
        with tc.tile_pool(name="xeT_p", bufs=1) as xeT_p, \
             tc.tile_pool(name="wih_p", bufs=1) as wih_p:
            wih_sb = wih_p.tile([128, QC, H], F16)
            nc.sync.dma_start(wih_sb[:], w_ihT.ap().rearrange("q p h -> p q h"))
            xeT_sb = xeT_p.tile([128, QC, NT], F16)
            with tc.tile_pool(name="xe_p", bufs=1) as xe_p, \
                 tc.tile_pool(name="idx_p", bufs=1) as idx_p, \
                 tc.tile_pool(name="trp_p", bufs=4, space="PSUM") as trp_p:
                xidx_sb = idx_p.tile([128, NT // 16], I16)
                nc.sync.dma_start(xidx_sb[:], x_idx[:, :])
                xe_sb = xe_p.tile([128, NT // 128, E], F16)
                GCH = 1024  # indices per dma_gather (keeps SWDGE ring within capacity)
                for g in range(NT // GCH):
                    nc.gpsimd.dma_gather(
                        out_ap=xe_sb[:, g * (GCH // 128):(g + 1) * (GCH // 128), :],
                        in_ap=emb_t.ap(),
                        idxs_ap=xidx_sb[:, g * (GCH // 16):(g + 1) * (GCH // 16)],
                        num_idxs=GCH, num_idxs_reg=GCH, elem_size=E)
                for c in range(NT // 128):
                    for q in range(QC):
                        pt = trp_p.tile([128, 128], F16)
                        nc.tensor.transpose(pt[:], xe_sb[:, c, q * 128:(q + 1) * 128], ident[:])
                        nc.vector.tensor_copy(xeT_sb[:, q, c * 128:(c + 1) * 128], pt[:])

            # ========= phase 3: xW^T[h, t] = W_ih @ xe^T + bias (hT layout) =========
            with tc.tile_pool(name="xw_ps", bufs=2, space="PSUM") as xw_ps, \
                 tc.tile_pool(name="xw_ev", bufs=3) as xw_ev:
                for hc in range(KC):
                    for tcn in range(NT // 512):
                        ps = xw_ps.tile([128, 512], FP)
                        for q in range(QC):
                            nc.tensor.matmul(
                                ps[:], wih_sb[:, q, hc * 128:(hc + 1) * 128],
                                xeT_sb[:, q, tcn * 512:(tcn + 1) * 512],
                                start=(q == 0), stop=(q == QC - 1))
                        ev = xw_ev.tile([128, 512], F16)
                        nc.vector.tensor_scalar_add(ev[:], ps[:], bias_pb_sb[:, hc:hc + 1])
                        nc.sync.dma_start(xw_dram[hc, tcn, :, :], ev[:])

        # W_out prefetch buffer (first WO_PRE k-slices, loaded during RNN)
        woA_p = top.enter_context(tc.tile_pool(name="woA_p", bufs=1))
        woA = woA_p.tile([128, WO_PRE, V_SH], F16)

        # h history, h-on-partition, s innermost: o2v[p, mc, b, s]
        # (scoped: freed before the projection phase needs its SBUF)
        o2v_stack = ExitStack()
        o2v_p = o2v_stack.enter_context(tc.tile_pool(name="o2v_p", bufs=1))
        o2v = o2v_p.tile([128, KC, B_LOC, S], F16)

        # W_out prefetch: issue early; DMA engines are idle during the RNN.
        for i, k in enumerate(WO_PRE_SLICES):
            nc.sync.dma_start(woA[:, i, :], w_outT[k, :, :])

        # ================= phase 4: RNN (weight-stationary, hT layout) =========
        # Wavefront (qf, kc-pair) emission: psum quarter 0 finishes early so
        # its tanh overlaps the tail of the step; consumers touch late h
        # quarters late, so the tanh chain pipelines across steps with no
        # exposed PE stall. tanh writes straight into o2v (no stage buffer).
        QW = 2 * B_LOC                # 16 cols per quarter
        WAVE = [(0, 0), (1, 0), (0, 1), (2, 0), (1, 1), (0, 2), (3, 0),
                (2, 1), (1, 2), (0, 3), (3, 1), (2, 2), (1, 3), (3, 2),
                (2, 3), (3, 3)]
        with tc.tile_pool(name="whh_p", bufs=1) as whh_p, \
             tc.tile_pool(name="hb_p", bufs=3) as hb_p, \
             tc.tile_pool(name="xwb_p", bufs=2) as xwb_p, \
             tc.tile_pool(name="rnn_ps", bufs=8, space="PSUM") as rnn_ps:
            whh_sb = whh_p.tile([128, KC, H], F16)
            nc.sync.dma_start(whh_sb[:], w_hhT.ap().rearrange("k p h -> p k h"))
            h0 = hb_p.tile([128, KC * B_LOC], F16, tag="hb")
            nc.vector.memset(h0[:], 0.0)
            hb_prev = h0
            xwb = None
            prev_mm = None
            for s in range(S):
                if s % 64 == 0:
                    blk = s // 64
                    xwb = xwb_p.tile([128, KC, 512], F16, tag="xwb")
                    nc.sync.dma_start(xwb[:], xw_dram[:, blk, :, :].rearrange(
                        "k p t -> p k t"))
                xw_sl = xwb[:].rearrange("p k (si b) -> p si k b", b=B_LOC)
                hb = hb_p.tile([128, KC * B_LOC], F16, tag="hb",
                               name=f"hb_{s}")
                ph = [None] * 4
                for qf, kp in WAVE:
                    if kp == 0:
                        ph[qf] = rnn_ps.tile([128, QW], FP, tag="rnn_psum",
                                             name=f"ph_{s}_{qf}")
                        m = nc.tensor.matmul(
                            ph[qf][:], ident[:],
                            xw_sl[:, s % 64, qf * 2:(qf + 1) * 2, :],
                            start=True, stop=False, skip_group_check=True)
                        if prev_mm is not None:
                            add_dep_helper(m.ins, prev_mm.ins, sync=False,
                                           reason="rnn wavefront order")
                        prev_mm = m
                    for kc in (kp * 2, kp * 2 + 1):
                        for mc in (qf * 2, qf * 2 + 1):
                            m = nc.tensor.matmul(
                                ph[qf][:, (mc - qf * 2) * B_LOC:(mc - qf * 2 + 1) * B_LOC],
                                whh_sb[:, kc, mc * 128:(mc + 1) * 128],
                                hb_prev[:, kc * B_LOC:(kc + 1) * B_LOC],
                                start=False,
                                stop=(kp == 3 and kc == kp * 2 + 1 and mc == qf * 2 + 1),
                                skip_group_check=True)
                            add_dep_helper(m.ins, prev_mm.ins, sync=False,
                                           reason="rnn wavefront order")
                            prev_mm = m
                    if kp == 3:
                        nc.scalar.activation(
                            hb[:, qf * QW:(qf + 1) * QW], ph[qf][:], Tanh)
                # h history copy for attention: DVE, off the critical path
                nc.vector.tensor_copy(
                    o2v[:, :, :, s],
                    hb[:].rearrange("p (m b) -> p m b", b=B_LOC))
                hb_prev = hb
            nc.vector.tensor_copy(lastT_sb[:], hb_prev[:])
        nc.sync.dma_start(agl_in[:, :], lastT_sb[:])
        if collective:
            nc.gpsimd.collective_compute(
                "AllGather", mybir.AluOpType.bypass,
                replica_groups=[list(range(n_cores))],
                ins=[agl_in.ap()], outs=[agl_out.ap()])
        else:
            for cc in range(n_cores):
                nc.sync.dma_start(agl_out[cc * 128:(cc + 1) * 128, :], agl_in[:, :])

        # ================= phase 5: attention =================
        with tc.tile_pool(name="att_ps", bufs=2, space="PSUM") as att_ps, \
             tc.tile_pool(name="sc_ps", bufs=4, space="PSUM") as sc_ps:
            # scores[b, s] = sum_h h_s[b,h]*last[b,h]: per (b, mc) 1-col matmuls
            scoresT = att_sb.tile([B_LOC, S], FP)
            with tc.tile_pool(name="scrow_p", bufs=2) as scrow_p:
                for b in range(B_LOC):
                    ps = sc_ps.tile([1, S], FP, tag="sc")
                    for mc in range(KC):
                        nc.tensor.matmul(
                            ps[:], lastT_sb[:, mc * B_LOC + b:mc * B_LOC + b + 1],
                            o2v[:, mc, b, :],
                            start=(mc == 0), stop=(mc == KC - 1))
                    row = scrow_p.tile([1, S], FP, tag="scrow")
                    nc.vector.tensor_copy(row[:], ps[:])
                    nc.sync.dma_start(scores_dram[0:1, b * S:(b + 1) * S], row[:])
            nc.sync.dma_start(
                scoresT[:, :],
                scores_dram.ap().rearrange("o (b s) -> (o b) s", b=B_LOC))

            # softmax over time (b on partitions); step S-1 excluded
            nc.vector.memset(scoresT[:, S - 1:S], -1e30)
            negmax = att_sb.tile([B_LOC, 1], FP)
            nc.vector.reduce_max(negmax[:], scoresT[:], axis=mybir.AxisListType.X, negate=True)
            expT = att_sb.tile([B_LOC, S], FP)
            nc.scalar.activation(expT[:], scoresT[:], Exp, bias=negmax[:])
            ssum = att_sb.tile([B_LOC, 1], FP)
            nc.vector.reduce_sum(ssum[:], expT[:], axis=mybir.AxisListType.X)
            rinv = att_sb.tile([B_LOC, 1], FP)
            nc.vector.reciprocal(rinv[:], ssum[:])
            attnT = att_sb.tile([B_LOC, S], F16)
            nc.vector.tensor_scalar_mul(attnT[:], expT[:], rinv[:])

            # broadcast attn over partitions: attnB[p, (b, s)] fp16
            nc.sync.dma_start(
                attn_dram.ap().rearrange("o (b s) -> (o b) s", b=B_LOC), attnT[:])
            attnB = att_sb.tile([128, B_LOC * S], F16)
            with tc.tile_pool(name="arow_p", bufs=2) as arow_p:
                for i in range(B_LOC * S // 512):
                    ar = arow_p.tile([1, 512], F16, tag="arow")
                    nc.sync.dma_start(ar[:], attn_dram[0:1, i * 512:(i + 1) * 512])
                    ab = att_ps.tile([128, 512], FP, tag="attps")
                    nc.tensor.matmul(ab[:], ones_row[:, 0:128], ar[:],
                                     start=True, stop=True)
                    nc.vector.tensor_copy(attnB[:, i * 512:(i + 1) * 512], ab[:])

            # att[b, h=mc*128+p]: per mc DVE mul + contiguous reduce over s
            att_acc = att_sb.tile([128, KC, B_LOC], FP)
            with tc.tile_pool(name="prod_p", bufs=2) as prod_p:
                for mc in range(KC):
                    pr = prod_p.tile([128, B_LOC * S], F16, tag="prod")
                    nc.vector.tensor_mul(pr[:], o2v[:, mc, :, :], attnB[:])
                    nc.vector.reduce_sum(
                        att_acc[:, mc, :],
                        pr[:].rearrange("p (b s) -> p b s", b=B_LOC),
                        axis=mybir.AxisListType.X)
            att16 = att_sb.tile([128, KC * B_LOC], F16)
            nc.vector.tensor_copy(
                att16[:].rearrange("p (m b) -> p m b", b=B_LOC), att_acc[:])
            nc.sync.dma_start(aga_in[:, :], att16[:])
        o2v_stack.close()
        if collective:
            nc.gpsimd.collective_compute(
                "AllGather", mybir.AluOpType.bypass,
                replica_groups=[list(range(n_cores))],
                ins=[aga_in.ap()], outs=[aga_out.ap()])
        else:
            for cc in range(n_cores):
                nc.sync.dma_start(aga_out[cc * 128:(cc + 1) * 128, :], aga_in[:, :])

        # ================= phase 6: projection =================
        # featT halves come straight from the gathered transposed layouts.
        NV = V_SH // 8  # 500-wide psum chunks
        with tc.tile_pool(name="fT_p", bufs=1) as fT_p, \
             tc.tile_pool(name="wo_p", bufs=4) as wo_p, \
             tc.tile_pool(name="y_ps", bufs=1, space="PSUM") as y_ps, \
             tc.tile_pool(name="y_sb_p", bufs=1) as y_sb_p:
            featT_l = fT_p.tile([128, KC, N_CORES, B_LOC], F16)
            nc.sync.dma_start(
                featT_l[:],
                agl_out.ap().rearrange("(c p) (m b) -> p m c b",
                                       p=128, m=KC, b=B_LOC))
            featT_a = fT_p.tile([128, KC, N_CORES, B_LOC], F16)
            nc.sync.dma_start(
                featT_a[:],
                aga_out.ap().rearrange("(c p) (m b) -> p m c b",
                                       p=128, m=KC, b=B_LOC))
            psums = [y_ps.tile([B, NV], FP, tag=f"y{n}", name=f"ypsum{n}")
                     for n in range(8)]
            # last half first (kc 8..15): ready after CC1, overlaps attention
            for kc in list(range(8, 16)) + list(range(8)):
                if kc in WO_PRE_SLICES:
                    wot = woA[:, WO_PRE_SLICES.index(kc), :]
                else:
                    wt = wo_p.tile([128, V_SH], F16, tag="wot")
                    nc.sync.dma_start(wt[:], w_outT[kc, :, :])
                    wot = wt[:]
                fT = (featT_l if kc >= 8 else featT_a)[
                    :, kc - 8 if kc >= 8 else kc, :, :].rearrange(
                    "p c b -> p (c b)")
                for n in range(8):
                    nc.tensor.matmul(psums[n][:], fT,
                                     wot[:, n * NV:(n + 1) * NV],
                                     start=(kc == 8), stop=False)
            for n in range(8):
                nc.tensor.matmul(psums[n][:], ones_row[:, 0:B],
                                 bout_sb[0:1, n * NV:(n + 1) * NV],
                                 start=False, stop=True)
            y_sb = y_sb_p.tile([B, V_SH], FP)
            for n in range(8):
                nc.vector.tensor_copy(y_sb[:, n * NV:(n + 1) * NV], psums[n][:])
            nc.sync.dma_start(y_out[:, :], y_sb[:])

    nc.compile()
    return nc


def host_prep(X, emb, W_ih, W_hh, b_ih, b_hh, W_out, b_out, S=S_FULL, n_cores=N_CORES):
    """Build the per-core input maps (sharding + fp16 layout prep on host)."""
    NT = S * B_LOC
    emb_f = np.ascontiguousarray(np.asarray(emb, np.float32).astype(np.float16))
    w_ihT = np.ascontiguousarray(
        np.asarray(W_ih, np.float32).T.astype(np.float16).reshape(QC, 128, H))
    w_hhT = np.ascontiguousarray(
        np.asarray(W_hh, np.float32).T.astype(np.float16).reshape(KC, 128, H))
    bias_pb = np.ascontiguousarray(
        (np.asarray(b_ih, np.float32) + np.asarray(b_hh, np.float32)).reshape(KC, 128).T)
    in_maps = []
    for c in range(n_cores):
        Xl = np.asarray(X[c * B_LOC:(c + 1) * B_LOC, :S])
        tok = Xl.T.reshape(-1)                        # t = s*B_LOC + b
        idx = np.zeros((128, NT // 16), np.int16)
        for g in range(8):
            idx[g * 16:(g + 1) * 16, :] = tok.reshape(NT // 16, 16).T
        Wo = np.asarray(W_out[c * V_SH:(c + 1) * V_SH, :], np.float32)
        w_outT = np.ascontiguousarray(Wo.T.astype(np.float16).reshape(16, 128, V_SH))
        in_maps.append({
            "x_idx": idx,
            "emb_t": emb_f,
            "w_ihT": w_ihT,
            "w_hhT": w_hhT,
            "bias_pb": bias_pb,
            "w_outT": w_outT,
            "b_out_sh": np.asarray(b_out[c * V_SH:(c + 1) * V_SH],
                                   np.float32).astype(np.float16).reshape(1, V_SH),
        })
    return in_maps


_NC_CACHE = {}


def kernel(X, emb, W_ih, W_hh, b_ih, b_hh, W_out, b_out):
    X = np.asarray(X)
    in_maps = host_prep(X, emb, W_ih, W_hh, b_ih, b_hh, W_out, b_out)
    if "nc" not in _NC_CACHE:
        _NC_CACHE["nc"] = build_nc()
    nc = _NC_CACHE["nc"]
    res = run_bass_kernel_spmd(nc, in_maps, list(range(N_CORES)))
    Y = np.concatenate([res.results[i]["y_out"] for i in range(N_CORES)], axis=1)
    return Y.astype(np.float32)


if __name__ == "__main__":
    import importlib.util
    spec = importlib.util.spec_from_file_location("reference", "/root/problem/reference.py")
    ref = importlib.util.module_from_spec(spec)
    spec.loader.exec_module(ref)
    inputs = {k: np.asarray(v) for k, v in ref.setup_inputs().items()}
    Y = kernel(**inputs)
    print(Y.shape, Y.dtype)
